# revision 4
# baseline (speedup 1.0000x reference)
"""Trainium2 Bass kernel for nn_AxonalConnections (gnn_message_passing).

Computes out[b,t] = sum_s adjacency[t,s] * mod[b,s],  mod = (1.5*E - 0.5) * spikes,
i.e. a batched mat-vec against a [16384, 16384] adjacency, reshaped to [32,128,128].

Sharding: adjacency row-shard (target dim) across 8 cores; spikes/E replicated;
each core produces out[:, t_shard] — pure output sharding, no collectives.

Two device paths:

* dense: bf16 GEMM, K=16384 accumulated in fp32 PSUM (fallback for arbitrary
  adjacency).

* sparse: when the adjacency's nonzeros all lie on the 9 conv-pattern
  diagonals (the generator's 3x3 message-passing graph), the GEMM is exactly a
  9-tap locally-connected stencil: out[b,t] = sum_k w9[t,k]*mod[b,t+d_k],
  evaluated on a [4 t-quarters x 32 batch, 512] packed layout (E-modulation
  folded into the weights on the host — exact, the factor is {1.0, -0.5}).

  v3 (all fp16):
  - The 9 tap offsets split into 3 groups of 3 with a uniform 128-element
    stride: {-128,0,128}+130 on the even-parity slab and {-129,-1,127}+129 /
    {-127,1,129}+129 on the odd-parity slab. Each group is ONE tensor_tensor
    with an overlapped [128, 3, 512] window AP (stride 128), so the whole
    stencil is 3 mults + 2 [128,3,512] adds + 2 small adds = 8 DVE
    instructions, all in fp16 2x_1P mode (even element offsets everywhere).
  - The spike slab ships in both parities (S0/S1, one element apart) so
    every window start is 4-byte aligned.
  - 6 taps ship batch-replicated as the two [128,3,512] group tensors on the
    ACT HWDGE ring (parallel to the SP ring); the last group (odd-B) ships
    compact ([4,512] rows + 0/1 selector) and is broadcast across batch
    lanes by TensorE (exact: one nonzero per column) + ScalarE PSUM->fp16
    copies, hiding it behind the first two groups' compute.
  - The final add runs in two halves, each immediately followed by its own
    output DMA, so the two HBM-write receipts overlap the tail instead of
    serializing after the last add.
"""

import sys

if "/opt/trn_rl_repo" not in sys.path:
    sys.path.insert(0, "/opt/trn_rl_repo")

from contextlib import ExitStack

import ml_dtypes
import numpy as np

B = 32
H = 128
W = 128
S = H * W            # 16384
NCORES = 8
TL = S // NCORES     # 2048 t-columns per core
KC = S // 128        # 128 contraction chunks (dense path)
P = 128

# sparse path geometry: 3x3 conv neighborhood offsets in flattened index space
DIAG_OFFSETS = [di * W + dj for di in (-1, 0, 1) for dj in (-1, 0, 1)]
NTAP = len(DIAG_OFFSETS)
NQ = 4               # t-quarters packed on partitions: 4*32 = 128
QT = TL // NQ        # 512 t per quarter
PADE = 130           # left pad of the spike slab
SW = 776             # per-parity slab width (max window start 258 + 512, even)
# tap groups with uniform window stride 128:
#   S0[i] = sp[tq + i - PADE]  -> tap d at offset 130+d   (even d)
#   S1[i] = S0[i+1]            -> tap d at offset 129+d   (odd d)
EVEN_TAPS = [1, 4, 7]   # d = -128, 0, +128  -> S0 offsets 2, 130, 258
ODDA_TAPS = [0, 3, 6]   # d = -129, -1, +127 -> S1 offsets 0, 128, 256
ODDB_TAPS = [2, 5, 8]   # d = -127, +1, +129 -> S1 offsets 2, 130, 258
WCW = 3 * QT + P        # compact odd-B rows + selector block

_progs = {}


def _build_dense():
    import concourse.tile as tile
    from concourse import bacc, mybir

    nc = bacc.Bacc("TRN2", target_bir_lowering=False, debug=False, num_devices=NCORES)
    f32 = mybir.dt.float32
    bf16 = mybir.dt.bfloat16

    adjt = nc.dram_tensor("adjt", [S, TL], bf16, kind="ExternalInput").ap()
    spt = nc.dram_tensor("spt", [P, KC, B], f32, kind="ExternalInput").ap()
    ef = nc.dram_tensor("ef", [P, KC], f32, kind="ExternalInput").ap()
    outt = nc.dram_tensor("out", [B, TL], f32, kind="ExternalOutput").ap()

    NT = TL // 512  # psum banks used for the output row block

    with tile.TileContext(nc) as tc:
        with ExitStack() as ctx:
            const = ctx.enter_context(tc.tile_pool(name="const", bufs=1))
            adj_pool = ctx.enter_context(tc.tile_pool(name="adj", bufs=10))
            psum = ctx.enter_context(tc.tile_pool(name="psum", bufs=1, space="PSUM"))
            outp = ctx.enter_context(tc.tile_pool(name="outp", bufs=1))

            sp_t = const.tile([P, KC, B], f32)
            nc.sync.dma_start(sp_t[:], spt[:])
            e_t = const.tile([P, KC], f32)
            nc.sync.dma_start(e_t[:], ef[:])
            fac = const.tile([P, KC], f32)
            # fac = 1.5*E - 0.5  (E in {0,1} -> {1.0, -0.5})
            nc.vector.tensor_scalar(
                fac[:], e_t[:], 1.5, -0.5,
                op0=mybir.AluOpType.mult, op1=mybir.AluOpType.add,
            )
            modt = const.tile([P, KC, B], bf16)
            for k in range(KC):
                nc.vector.tensor_scalar(
                    modt[:, k, :], sp_t[:, k, :], fac[:, k : k + 1], None,
                    op0=mybir.AluOpType.mult,
                )

            pts = [psum.tile([B, 512], f32, name=f"acc{j}") for j in range(NT)]
            for k in range(KC):
                at = adj_pool.tile([P, TL], bf16)
                nc.sync.dma_start(at[:], adjt[k * P : (k + 1) * P, :])
                for j in range(NT):
                    nc.tensor.matmul(
                        pts[j][:],
                        modt[:, k, :],
                        at[:, j * 512 : (j + 1) * 512],
                        start=(k == 0),
                        stop=(k == KC - 1),
                    )

            ot = outp.tile([B, TL], f32)
            for j in range(NT):
                nc.vector.tensor_copy(out=ot[:, j * 512 : (j + 1) * 512], in_=pts[j][:])
            nc.sync.dma_start(outt[:], ot[:])

    nc.compile()
    return nc


def _win3(t, base):
    """Overlapped [128, 3, 512] window AP over slab tile t: windows at
    element offsets base + {0, 128, 256}, each 512 long (stride-128 middle
    dim; every window start stays 4-byte aligned for the fp16 2x mode)."""
    from concourse.ap import AP

    a = t[:]
    part = list(list(p) for p in a.ap)[0]
    return AP(a.tensor, base, [part, [128, 3], [1, QT]])


def _build_sparse():
    import concourse.tile as tile
    from concourse import bacc, mybir

    nc = bacc.Bacc("TRN2", target_bir_lowering=False, debug=False, num_devices=NCORES)
    f16 = mybir.dt.float16
    f32 = mybir.dt.float32

    # per-core inputs (host pre-packed, fp16):
    s0d = nc.dram_tensor("s0", [P, SW], f16, kind="ExternalInput").ap()
    s1d = nc.dram_tensor("s1", [P, SW], f16, kind="ExternalInput").ap()
    wred = nc.dram_tensor("wre", [P, 3, QT], f16, kind="ExternalInput").ap()
    wrod = nc.dram_tensor("wro", [P, 3, QT], f16, kind="ExternalInput").ap()
    wcsd = nc.dram_tensor("wcs", [4, WCW], f16, kind="ExternalInput").ap()
    # packed [32q+b, t] layout; host unpacks to [B, TL]
    outt = nc.dram_tensor("out", [P, QT], f16, kind="ExternalOutput").ap()

    HQ = QT // 2

    with tile.TileContext(nc) as tc:
        with ExitStack() as ctx:
            pool = ctx.enter_context(tc.tile_pool(name="pool", bufs=1))
            psum = ctx.enter_context(tc.tile_pool(name="psum", bufs=1, space="PSUM"))

            # SP ring: spike slabs (even-parity first: its group computes
            # first) + the tiny compact/selector block; output later.
            s0 = pool.tile([P, SW], f16)
            nc.sync.dma_start(s0[:], s0d[:])
            s1 = pool.tile([P, SW], f16)
            nc.sync.dma_start(s1[:], s1d[:])
            wcs = pool.tile([4, WCW], f16)
            nc.sync.dma_start(wcs[:], wcsd[:])
            # ACT ring (drains in parallel with the SP ring): the two
            # DMA-replicated weight groups.
            wre = pool.tile([P, 3, QT], f16)
            nc.scalar.dma_start(wre[:], wred[:])
            wro = pool.tile([P, 3, QT], f16)
            nc.scalar.dma_start(wro[:], wrod[:])

            # broadcast the odd-B group across batch lanes:
            # psum[p, t] = sum_q sel[q, p] * wcs[q, j*QT+t]
            sel = wcs[:, 3 * QT : 3 * QT + P]
            wob = pool.tile([P, 3, QT], f16)
            for j in range(3):
                ps = psum.tile([P, QT], f32, name=f"bc{j}")
                nc.tensor.matmul(
                    ps[:], sel, wcs[:, j * QT : (j + 1) * QT],
                    start=True, stop=True,
                )
                nc.scalar.copy(wob[:, j, :], ps[:])

            mult = mybir.AluOpType.mult
            add = mybir.AluOpType.add
            pe = pool.tile([P, 3, QT], f16)
            nc.vector.tensor_tensor(pe[:], _win3(s0, 2), wre[:], mult)
            poa = pool.tile([P, 3, QT], f16)
            nc.vector.tensor_tensor(poa[:], _win3(s1, 0), wro[:], mult)
            r1 = pool.tile([P, 3, QT], f16)
            nc.vector.tensor_tensor(r1[:], pe[:], poa[:], add)
            pob = pool.tile([P, 3, QT], f16)
            nc.vector.tensor_tensor(pob[:], _win3(s1, 2), wob[:], mult)
            r2 = pool.tile([P, 3, QT], f16)
            nc.vector.tensor_tensor(r2[:], r1[:], pob[:], add)
            tt = pool.tile([P, QT], f16)
            nc.vector.tensor_tensor(tt[:], r2[:, 0, :], r2[:, 1, :], add)
            # final add + store in halves so the two HBM-write receipts overlap
            acc = pool.tile([P, QT], f16)
            for h in range(2):
                lo, hi = h * HQ, (h + 1) * HQ
                nc.vector.tensor_tensor(
                    acc[:, lo:hi], tt[:, lo:hi], r2[:, 2, lo:hi], add
                )
                nc.sync.dma_start(outt[:, lo:hi], acc[:, lo:hi])

    nc.compile()
    return nc


def _get_prog(name):
    if name not in _progs:
        _progs[name] = {"dense": _build_dense, "sparse": _build_sparse}[name]()
    return _progs[name]


def _run(nc, in_maps, **kwargs):
    from concourse.bass_utils import run_bass_kernel_spmd

    return run_bass_kernel_spmd(nc, in_maps, core_ids=list(range(NCORES)), **kwargs)


def _extract_diagonals(adjacency):
    """W9[t, k] = adjacency[t, t + d_k] (0 where out of range).

    Returns (W9, exact) where exact means every nonzero of adjacency lies on
    those 9 diagonals, making the stencil reproduction of the GEMM exact.
    """
    t = np.arange(S)
    W9 = np.zeros((S, NTAP), np.float32)
    for k, d in enumerate(DIAG_OFFSETS):
        s = t + d
        valid = (s >= 0) & (s < S)
        W9[valid, k] = adjacency[t[valid], s[valid]]
    exact = np.count_nonzero(adjacency) == np.count_nonzero(W9)
    return W9, exact


def _prep_dense_inmaps(sp_flat, E_flat, adjacency):
    spt = np.ascontiguousarray(sp_flat.T.reshape(KC, P, B).transpose(1, 0, 2))
    ef = np.ascontiguousarray(E_flat.reshape(KC, P).T)
    adj_bf = adjacency.astype(ml_dtypes.bfloat16)
    in_maps = []
    for m in range(NCORES):
        adjt_m = np.ascontiguousarray(adj_bf[m * TL : (m + 1) * TL, :].T)
        in_maps.append({"adjt": adjt_m, "spt": spt, "ef": ef})
    return in_maps


def _group_w(wslab, taps):
    # [NQ, QT, NTAP] -> batch-replicated [P, 3, QT] for the given tap trio
    g = wslab[:, :, taps].transpose(0, 2, 1)            # [NQ, 3, QT]
    g = np.broadcast_to(g[:, None], (NQ, B, 3, QT))
    return np.ascontiguousarray(g).reshape(P, 3, QT)


def _prep_sparse_inmaps(sp_flat, E_flat, W9):
    # fold the E-modulation into the tap weights: exact because the factor is
    # the power-of-two scale {1.0, -0.5}
    fac = 1.5 * E_flat - 0.5
    t = np.arange(S)
    wfold = np.empty_like(W9)  # [S, 9]
    for k, d in enumerate(DIAG_OFFSETS):
        s = np.clip(t + d, 0, S - 1)
        wfold[:, k] = W9[:, k] * fac[s]
    wfold = wfold.astype(np.float16)

    sp_pad = np.zeros((B, S + 2 * PADE + 8), np.float16)
    sp_pad[:, PADE : PADE + S] = sp_flat

    # 0/1 selector shared across cores: sel[q, p] = (p // 32 == q)
    sel = (np.arange(P)[None, :] // B == np.arange(NQ)[:, None]).astype(np.float16)

    in_maps = []
    for m in range(NCORES):
        t0 = m * TL
        s0 = np.empty((NQ, B, SW), np.float16)
        s1 = np.empty((NQ, B, SW), np.float16)
        for q in range(NQ):
            tq = t0 + q * QT
            s0[q] = sp_pad[:, tq : tq + SW]
            s1[q] = sp_pad[:, tq + 1 : tq + 1 + SW]

        wslab = wfold[t0 : t0 + TL].reshape(NQ, QT, NTAP)
        wcs = np.empty((NQ, WCW), np.float16)
        wcs[:, : 3 * QT] = (
            wslab[:, :, ODDB_TAPS].transpose(0, 2, 1).reshape(NQ, 3 * QT)
        )
        wcs[:, 3 * QT :] = sel
        in_maps.append(
            {
                "s0": s0.reshape(P, SW),
                "s1": s1.reshape(P, SW),
                "wre": _group_w(wslab, EVEN_TAPS),
                "wro": _group_w(wslab, ODDA_TAPS),
                "wcs": wcs,
            }
        )
    return in_maps


def _gather_out(results):
    out = np.empty((B, S), np.float32)
    for m in range(NCORES):
        r = results[m]["out"]
        if r.shape == (P, QT):  # sparse path: unpack [32q+b, t] -> [b, q*QT+t]
            r = r.astype(np.float32).reshape(NQ, B, QT).transpose(1, 0, 2)
            r = r.reshape(B, TL)
        out[:, m * TL : (m + 1) * TL] = r
    return out


def kernel(spikes, E, adjacency):
    spikes = np.asarray(spikes, np.float32)
    E = np.asarray(E, np.float32)
    adjacency = np.asarray(adjacency, np.float32)
    sp_flat = spikes.reshape(B, S)
    E_flat = E.reshape(S)

    W9, exact = _extract_diagonals(adjacency)
    if exact:
        in_maps = _prep_sparse_inmaps(sp_flat, E_flat, W9)
        results = _run(_get_prog("sparse"), in_maps).results
    else:
        in_maps = _prep_dense_inmaps(sp_flat, E_flat, adjacency)
        results = _run(_get_prog("dense"), in_maps).results
    return _gather_out(results).reshape(B, H, W)


# revision 5
# speedup vs baseline: 1.0606x; 1.0606x over previous
"""Trainium2 Bass kernel for nn_AxonalConnections (gnn_message_passing).

Computes out[b,t] = sum_s adjacency[t,s] * mod[b,s],  mod = (1.5*E - 0.5) * spikes,
i.e. a batched mat-vec against a [16384, 16384] adjacency, reshaped to [32,128,128].

Sharding: adjacency row-shard (target dim) across 8 cores; spikes/E replicated;
each core produces out[:, t_shard] — pure output sharding, no collectives.

Two device paths:

* dense: bf16 GEMM, K=16384 accumulated in fp32 PSUM (fallback for arbitrary
  adjacency).

* sparse: when the adjacency's nonzeros all lie on the 9 conv-pattern
  diagonals (the generator's 3x3 message-passing graph), the GEMM is exactly a
  9-tap locally-connected stencil: out[b,t] = sum_k w9[t,k]*mod[b,t+d_k],
  evaluated on a [4 t-quarters x 32 batch, 512] packed layout (E-modulation
  folded into the weights on the host — exact, the factor is {1.0, -0.5}).

  v4 (all fp16, pure DVE pipeline):
  - The whole stencil runs as 9 tensor_tensor mults + 8 adds in fp16 so the
    DVE hits its 2x_1P mode (~420 ns per [128,512] op, measured). Tap
    offsets have mixed parity, so the padded spike slab ships in both
    parities (S0/S1, one element apart) and each tap reads whichever copy
    makes its window start 4-byte aligned.
  - Profiling showed the body is gated by DMA *landing latency*, not DMA
    bytes: a tensor's completion semaphore fires only when the last SDMA
    engine finishes it, so big monolithic transfers (and few-partition
    tensors, which serialize on a single engine) delay the first compute by
    ~5 us. So every tap's batch-replicated weights ship as their own
    [128, 512] DMA, split across the two HWDGE rings (SP + ACT) in exactly
    the order the DVE consumes them; the DVE starts ~2.5 us earlier and
    never starves.
  - The final add runs in two halves, each immediately followed by its own
    output DMA, so the two HBM-write receipts overlap the tail instead of
    serializing after the last add.
"""

import sys

if "/opt/trn_rl_repo" not in sys.path:
    sys.path.insert(0, "/opt/trn_rl_repo")

from contextlib import ExitStack

import ml_dtypes
import numpy as np

B = 32
H = 128
W = 128
S = H * W            # 16384
NCORES = 8
TL = S // NCORES     # 2048 t-columns per core
KC = S // 128        # 128 contraction chunks (dense path)
P = 128

# sparse path geometry: 3x3 conv neighborhood offsets in flattened index space
DIAG_OFFSETS = [di * W + dj for di in (-1, 0, 1) for dj in (-1, 0, 1)]
NTAP = len(DIAG_OFFSETS)
NQ = 4               # t-quarters packed on partitions: 4*32 = 128
QT = TL // NQ        # 512 t per quarter
PADE = 130           # left pad of the spike slab
SW = 776             # per-parity slab width (max window start 258 + 512, even)
# S0[i] = sp[tq + i - PADE] -> tap d at even offset 130+d for even d
# S1[i] = S0[i+1]           -> tap d at even offset 129+d for odd d
# DVE consumption order: even-d taps first (S0 + first weight DMAs), then the
# rest in ring-landing order.
TAP_ORDER = [1, 4, 7, 0, 3, 6, 2, 5, 8]
ACT_TAPS = [1, 4, 7, 0, 3, 6]   # taps shipped on the ACT HWDGE ring
SYNC_TAPS = [2, 5, 8]           # taps shipped on the SP ring after the slabs

_progs = {}


def _build_dense():
    import concourse.tile as tile
    from concourse import bacc, mybir

    nc = bacc.Bacc("TRN2", target_bir_lowering=False, debug=False, num_devices=NCORES)
    f32 = mybir.dt.float32
    bf16 = mybir.dt.bfloat16

    adjt = nc.dram_tensor("adjt", [S, TL], bf16, kind="ExternalInput").ap()
    spt = nc.dram_tensor("spt", [P, KC, B], f32, kind="ExternalInput").ap()
    ef = nc.dram_tensor("ef", [P, KC], f32, kind="ExternalInput").ap()
    outt = nc.dram_tensor("out", [B, TL], f32, kind="ExternalOutput").ap()

    NT = TL // 512  # psum banks used for the output row block

    with tile.TileContext(nc) as tc:
        with ExitStack() as ctx:
            const = ctx.enter_context(tc.tile_pool(name="const", bufs=1))
            adj_pool = ctx.enter_context(tc.tile_pool(name="adj", bufs=10))
            psum = ctx.enter_context(tc.tile_pool(name="psum", bufs=1, space="PSUM"))
            outp = ctx.enter_context(tc.tile_pool(name="outp", bufs=1))

            sp_t = const.tile([P, KC, B], f32)
            nc.sync.dma_start(sp_t[:], spt[:])
            e_t = const.tile([P, KC], f32)
            nc.sync.dma_start(e_t[:], ef[:])
            fac = const.tile([P, KC], f32)
            # fac = 1.5*E - 0.5  (E in {0,1} -> {1.0, -0.5})
            nc.vector.tensor_scalar(
                fac[:], e_t[:], 1.5, -0.5,
                op0=mybir.AluOpType.mult, op1=mybir.AluOpType.add,
            )
            modt = const.tile([P, KC, B], bf16)
            for k in range(KC):
                nc.vector.tensor_scalar(
                    modt[:, k, :], sp_t[:, k, :], fac[:, k : k + 1], None,
                    op0=mybir.AluOpType.mult,
                )

            pts = [psum.tile([B, 512], f32, name=f"acc{j}") for j in range(NT)]
            for k in range(KC):
                at = adj_pool.tile([P, TL], bf16)
                nc.sync.dma_start(at[:], adjt[k * P : (k + 1) * P, :])
                for j in range(NT):
                    nc.tensor.matmul(
                        pts[j][:],
                        modt[:, k, :],
                        at[:, j * 512 : (j + 1) * 512],
                        start=(k == 0),
                        stop=(k == KC - 1),
                    )

            ot = outp.tile([B, TL], f32)
            for j in range(NT):
                nc.vector.tensor_copy(out=ot[:, j * 512 : (j + 1) * 512], in_=pts[j][:])
            nc.sync.dma_start(outt[:], ot[:])

    nc.compile()
    return nc


def _tap_slice(s0, s1, d):
    """AP slice of the dual-parity spike slabs for tap offset d (start even)."""
    if d % 2 == 0:
        return s0[:, 130 + d : 130 + d + QT]
    return s1[:, 129 + d : 129 + d + QT]


def _build_sparse():
    import concourse.tile as tile
    from concourse import bacc, mybir

    nc = bacc.Bacc("TRN2", target_bir_lowering=False, debug=False, num_devices=NCORES)
    f16 = mybir.dt.float16

    s0d = nc.dram_tensor("s0", [P, SW], f16, kind="ExternalInput").ap()
    s1d = nc.dram_tensor("s1", [P, SW], f16, kind="ExternalInput").ap()
    wd = [
        nc.dram_tensor(f"w{k}", [P, QT], f16, kind="ExternalInput").ap()
        for k in range(NTAP)
    ]
    # packed [32q+b, t] layout; host unpacks to [B, TL]
    outt = nc.dram_tensor("out", [P, QT], f16, kind="ExternalOutput").ap()

    HQ = QT // 2

    with tile.TileContext(nc) as tc:
        with ExitStack() as ctx:
            pool = ctx.enter_context(tc.tile_pool(name="pool", bufs=1))

            # SP ring: the two spike slabs, then the last-consumed taps.
            # ACT ring (drains in parallel): the first-consumed taps.
            # Every tensor is its own DMA so its completion semaphore fires
            # as soon as its own bytes land.
            s0 = pool.tile([P, SW], f16)
            nc.sync.dma_start(s0[:], s0d[:])
            s1 = pool.tile([P, SW], f16)
            nc.sync.dma_start(s1[:], s1d[:])
            wt = {}
            for k in ACT_TAPS:
                wt[k] = pool.tile([P, QT], f16, name=f"w{k}")
                nc.scalar.dma_start(wt[k][:], wd[k][:])
            for k in SYNC_TAPS:
                wt[k] = pool.tile([P, QT], f16, name=f"w{k}")
                nc.sync.dma_start(wt[k][:], wd[k][:])

            mult = mybir.AluOpType.mult
            add = mybir.AluOpType.add
            acc = None
            for i, k in enumerate(TAP_ORDER):
                d = DIAG_OFFSETS[k]
                sh = _tap_slice(s0, s1, d)
                prod = pool.tile([P, QT], f16, name=f"prod{k}")
                nc.vector.tensor_tensor(prod[:], sh, wt[k][:], mult)
                if acc is None:
                    acc = prod
                elif i < NTAP - 1:
                    nxt = pool.tile([P, QT], f16, name=f"acc{i}")
                    nc.vector.tensor_tensor(nxt[:], acc[:], prod[:], add)
                    acc = nxt
                else:
                    # final add + store in halves so the two HBM-write
                    # receipts overlap
                    fin = pool.tile([P, QT], f16, name="fin")
                    for h in range(2):
                        lo, hi = h * HQ, (h + 1) * HQ
                        nc.vector.tensor_tensor(
                            fin[:, lo:hi], acc[:, lo:hi], prod[:, lo:hi], add
                        )
                        nc.sync.dma_start(outt[:, lo:hi], fin[:, lo:hi])

    nc.compile()
    return nc


def _get_prog(name):
    if name not in _progs:
        _progs[name] = {"dense": _build_dense, "sparse": _build_sparse}[name]()
    return _progs[name]


def _run(nc, in_maps, **kwargs):
    from concourse.bass_utils import run_bass_kernel_spmd

    return run_bass_kernel_spmd(nc, in_maps, core_ids=list(range(NCORES)), **kwargs)


def _extract_diagonals(adjacency):
    """W9[t, k] = adjacency[t, t + d_k] (0 where out of range).

    Returns (W9, exact) where exact means every nonzero of adjacency lies on
    those 9 diagonals, making the stencil reproduction of the GEMM exact.
    """
    t = np.arange(S)
    W9 = np.zeros((S, NTAP), np.float32)
    for k, d in enumerate(DIAG_OFFSETS):
        s = t + d
        valid = (s >= 0) & (s < S)
        W9[valid, k] = adjacency[t[valid], s[valid]]
    exact = np.count_nonzero(adjacency) == np.count_nonzero(W9)
    return W9, exact


def _prep_dense_inmaps(sp_flat, E_flat, adjacency):
    spt = np.ascontiguousarray(sp_flat.T.reshape(KC, P, B).transpose(1, 0, 2))
    ef = np.ascontiguousarray(E_flat.reshape(KC, P).T)
    adj_bf = adjacency.astype(ml_dtypes.bfloat16)
    in_maps = []
    for m in range(NCORES):
        adjt_m = np.ascontiguousarray(adj_bf[m * TL : (m + 1) * TL, :].T)
        in_maps.append({"adjt": adjt_m, "spt": spt, "ef": ef})
    return in_maps


def _prep_sparse_inmaps(sp_flat, E_flat, W9):
    # fold the E-modulation into the tap weights: exact because the factor is
    # the power-of-two scale {1.0, -0.5}
    fac = 1.5 * E_flat - 0.5
    t = np.arange(S)
    wfold = np.empty_like(W9)  # [S, 9]
    for k, d in enumerate(DIAG_OFFSETS):
        s = np.clip(t + d, 0, S - 1)
        wfold[:, k] = W9[:, k] * fac[s]
    wfold = wfold.astype(np.float16)

    sp_pad = np.zeros((B, S + 2 * PADE + 8), np.float16)
    sp_pad[:, PADE : PADE + S] = sp_flat

    in_maps = []
    for m in range(NCORES):
        t0 = m * TL
        s0 = np.empty((NQ, B, SW), np.float16)
        s1 = np.empty((NQ, B, SW), np.float16)
        for q in range(NQ):
            tq = t0 + q * QT
            s0[q] = sp_pad[:, tq : tq + SW]
            s1[q] = sp_pad[:, tq + 1 : tq + 1 + SW]

        wslab = wfold[t0 : t0 + TL].reshape(NQ, QT, NTAP)
        im = {"s0": s0.reshape(P, SW), "s1": s1.reshape(P, SW)}
        for k in range(NTAP):
            wk = np.broadcast_to(wslab[:, None, :, k], (NQ, B, QT))
            im[f"w{k}"] = np.ascontiguousarray(wk).reshape(P, QT)
        in_maps.append(im)
    return in_maps


def _gather_out(results):
    out = np.empty((B, S), np.float32)
    for m in range(NCORES):
        r = results[m]["out"]
        if r.shape == (P, QT):  # sparse path: unpack [32q+b, t] -> [b, q*QT+t]
            r = r.astype(np.float32).reshape(NQ, B, QT).transpose(1, 0, 2)
            r = r.reshape(B, TL)
        out[:, m * TL : (m + 1) * TL] = r
    return out


def kernel(spikes, E, adjacency):
    spikes = np.asarray(spikes, np.float32)
    E = np.asarray(E, np.float32)
    adjacency = np.asarray(adjacency, np.float32)
    sp_flat = spikes.reshape(B, S)
    E_flat = E.reshape(S)

    W9, exact = _extract_diagonals(adjacency)
    if exact:
        in_maps = _prep_sparse_inmaps(sp_flat, E_flat, W9)
        results = _run(_get_prog("sparse"), in_maps).results
    else:
        in_maps = _prep_dense_inmaps(sp_flat, E_flat, adjacency)
        results = _run(_get_prog("dense"), in_maps).results
    return _gather_out(results).reshape(B, H, W)


# revision 9
# speedup vs baseline: 1.1210x; 1.0569x over previous
"""Trainium2 Bass kernel for nn_AxonalConnections (gnn_message_passing).

Computes out[b,t] = sum_s adjacency[t,s] * mod[b,s],  mod = (1.5*E - 0.5) * spikes,
i.e. a batched mat-vec against a [16384, 16384] adjacency, reshaped to [32,128,128].

Sharding: adjacency row-shard (target dim) across 8 cores; spikes/E replicated;
each core produces out[:, t_shard] — pure output sharding, no collectives.

Two device paths:

* dense: bf16 GEMM, K=16384 accumulated in fp32 PSUM (fallback for arbitrary
  adjacency).

* sparse: when the adjacency's nonzeros all lie on the 9 conv-pattern
  diagonals (the generator's 3x3 message-passing graph), the GEMM is exactly a
  9-tap locally-connected stencil: out[b,t] = sum_k w9[t,k]*mod[b,t+d_k],
  evaluated on a [4 t-quarters x 32 batch, 512] packed layout (E-modulation
  folded into the weights on the host — exact, the factor is {1.0, -0.5}).

  v4 (all fp16, pure DVE pipeline):
  - The whole stencil runs as 9 tensor_tensor mults + 8 adds in fp16 so the
    DVE hits its 2x_1P mode (~420 ns per [128,512] op, measured). Tap
    offsets have mixed parity, so the padded spike slab ships in both
    parities (S0/S1, one element apart) and each tap reads whichever copy
    makes its window start 4-byte aligned.
  - Profiling showed the body is gated by DMA *landing latency*, not DMA
    bytes: a tensor's completion semaphore fires only when the last SDMA
    engine finishes it, so big monolithic transfers (and few-partition
    tensors, which serialize on a single engine) delay the first compute by
    ~5 us. So every tap's batch-replicated weights ship as their own
    [128, 512] DMA, split across the two HWDGE rings (SP + ACT) in exactly
    the order the DVE consumes them; the DVE starts ~2.5 us earlier and
    never starves.
  - The final add runs in two halves, each immediately followed by its own
    output DMA, so the two HBM-write receipts overlap the tail instead of
    serializing after the last add.
"""

import sys

if "/opt/trn_rl_repo" not in sys.path:
    sys.path.insert(0, "/opt/trn_rl_repo")

from contextlib import ExitStack

import ml_dtypes
import numpy as np

B = 32
H = 128
W = 128
S = H * W            # 16384
NCORES = 8
TL = S // NCORES     # 2048 t-columns per core
KC = S // 128        # 128 contraction chunks (dense path)
P = 128

# sparse path geometry: 3x3 conv neighborhood offsets in flattened index space
DIAG_OFFSETS = [di * W + dj for di in (-1, 0, 1) for dj in (-1, 0, 1)]
NTAP = len(DIAG_OFFSETS)
NQ = 4               # t-quarters packed on partitions: 4*32 = 128
QT = TL // NQ        # 512 t per quarter
PADE = 130           # left pad of the spike slab
SW = 776             # per-parity slab width (max window start 258 + 512, even)
# S0[i] = sp[tq + i - PADE] -> tap d at even offset 130+d for even d
# S1[i] = S0[i+1]           -> tap d at even offset 129+d for odd d
# DVE consumption order: even-d taps first (S0 lands first), then the rest.
# Taps ship as PAIR tensors (plus one single) so the total DMA count stays
# under the Tile scheduler's 8 completion-semaphore lanes, striped across the
# two HWDGE rings in consumption order so completions arrive just in time.
TAP_ORDER = [1, 4, 7, 0, 3, 6, 2, 5, 8]
WGROUPS = [(1, 4), (7, 0), (3, 6), (2, 5), (8,)]
WG_RING = ["act", "sync", "act", "sync", "act"]

_progs = {}


def _build_dense():
    import concourse.tile as tile
    from concourse import bacc, mybir

    nc = bacc.Bacc("TRN2", target_bir_lowering=False, debug=False, num_devices=NCORES)
    f32 = mybir.dt.float32
    bf16 = mybir.dt.bfloat16

    adjt = nc.dram_tensor("adjt", [S, TL], bf16, kind="ExternalInput").ap()
    spt = nc.dram_tensor("spt", [P, KC, B], f32, kind="ExternalInput").ap()
    ef = nc.dram_tensor("ef", [P, KC], f32, kind="ExternalInput").ap()
    outt = nc.dram_tensor("out", [B, TL], f32, kind="ExternalOutput").ap()

    NT = TL // 512  # psum banks used for the output row block

    with tile.TileContext(nc) as tc:
        with ExitStack() as ctx:
            const = ctx.enter_context(tc.tile_pool(name="const", bufs=1))
            adj_pool = ctx.enter_context(tc.tile_pool(name="adj", bufs=10))
            psum = ctx.enter_context(tc.tile_pool(name="psum", bufs=1, space="PSUM"))
            outp = ctx.enter_context(tc.tile_pool(name="outp", bufs=1))

            sp_t = const.tile([P, KC, B], f32)
            nc.sync.dma_start(sp_t[:], spt[:])
            e_t = const.tile([P, KC], f32)
            nc.sync.dma_start(e_t[:], ef[:])
            fac = const.tile([P, KC], f32)
            # fac = 1.5*E - 0.5  (E in {0,1} -> {1.0, -0.5})
            nc.vector.tensor_scalar(
                fac[:], e_t[:], 1.5, -0.5,
                op0=mybir.AluOpType.mult, op1=mybir.AluOpType.add,
            )
            modt = const.tile([P, KC, B], bf16)
            for k in range(KC):
                nc.vector.tensor_scalar(
                    modt[:, k, :], sp_t[:, k, :], fac[:, k : k + 1], None,
                    op0=mybir.AluOpType.mult,
                )

            pts = [psum.tile([B, 512], f32, name=f"acc{j}") for j in range(NT)]
            for k in range(KC):
                at = adj_pool.tile([P, TL], bf16)
                nc.sync.dma_start(at[:], adjt[k * P : (k + 1) * P, :])
                for j in range(NT):
                    nc.tensor.matmul(
                        pts[j][:],
                        modt[:, k, :],
                        at[:, j * 512 : (j + 1) * 512],
                        start=(k == 0),
                        stop=(k == KC - 1),
                    )

            ot = outp.tile([B, TL], f32)
            for j in range(NT):
                nc.vector.tensor_copy(out=ot[:, j * 512 : (j + 1) * 512], in_=pts[j][:])
            nc.sync.dma_start(outt[:], ot[:])

    nc.compile()
    return nc


def _tap_slice(s0, s1, d):
    """AP slice of the dual-parity spike slabs for tap offset d (start even)."""
    if d % 2 == 0:
        return s0[:, 130 + d : 130 + d + QT]
    return s1[:, 129 + d : 129 + d + QT]


def _build_sparse():
    import concourse.tile as tile
    from concourse import bacc, mybir

    nc = bacc.Bacc("TRN2", target_bir_lowering=False, debug=False, num_devices=NCORES)
    f16 = mybir.dt.float16

    s0d = nc.dram_tensor("s0", [P, SW], f16, kind="ExternalInput").ap()
    s1d = nc.dram_tensor("s1", [P, SW], f16, kind="ExternalInput").ap()
    wgd = [
        nc.dram_tensor(f"wg{g}", [P, len(ks), QT], f16, kind="ExternalInput").ap()
        for g, ks in enumerate(WGROUPS)
    ]
    # packed [32q+b, t] layout; host unpacks to [B, TL]
    outt = nc.dram_tensor("out", [P, QT], f16, kind="ExternalOutput").ap()

    HQ = QT // 2

    with tile.TileContext(nc) as tc:
        with ExitStack() as ctx:
            pool = ctx.enter_context(tc.tile_pool(name="pool", bufs=1))

            # SP ring: the spike slabs then alternate weight groups; ACT ring
            # (drains in parallel) the other groups — striped in DVE
            # consumption order so each group's completion semaphore fires
            # just before its taps are consumed.
            s0 = pool.tile([P, SW], f16)
            nc.sync.dma_start(s0[:], s0d[:])
            s1 = pool.tile([P, SW], f16)
            nc.sync.dma_start(s1[:], s1d[:])
            wt = {}
            for g, ks in enumerate(WGROUPS):
                wg = pool.tile([P, len(ks), QT], f16, name=f"wg{g}")
                eng = nc.scalar if WG_RING[g] == "act" else nc.sync
                eng.dma_start(wg[:], wgd[g][:])
                for j, k in enumerate(ks):
                    wt[k] = wg[:, j, :]

            mult = mybir.AluOpType.mult
            add = mybir.AluOpType.add
            acc = None
            for i, k in enumerate(TAP_ORDER):
                d = DIAG_OFFSETS[k]
                sh = _tap_slice(s0, s1, d)
                prod = pool.tile([P, QT], f16, name=f"prod{k}")
                nc.vector.tensor_tensor(prod[:], sh, wt[k], mult)
                if acc is None:
                    acc = prod
                elif i < NTAP - 1:
                    nxt = pool.tile([P, QT], f16, name=f"acc{i}")
                    nc.vector.tensor_tensor(nxt[:], acc[:], prod[:], add)
                    acc = nxt
                else:
                    # final add + store in halves so the two HBM-write
                    # receipts overlap
                    fin = pool.tile([P, QT], f16, name="fin")
                    for h in range(2):
                        lo, hi = h * HQ, (h + 1) * HQ
                        nc.vector.tensor_tensor(
                            fin[:, lo:hi], acc[:, lo:hi], prod[:, lo:hi], add
                        )
                        nc.sync.dma_start(outt[:, lo:hi], fin[:, lo:hi])

    nc.compile()
    return nc


def _get_prog(name):
    if name not in _progs:
        _progs[name] = {"dense": _build_dense, "sparse": _build_sparse}[name]()
    return _progs[name]


def _run(nc, in_maps, **kwargs):
    from concourse.bass_utils import run_bass_kernel_spmd

    return run_bass_kernel_spmd(nc, in_maps, core_ids=list(range(NCORES)), **kwargs)


def _extract_diagonals(adjacency):
    """W9[t, k] = adjacency[t, t + d_k] (0 where out of range).

    Returns (W9, exact) where exact means every nonzero of adjacency lies on
    those 9 diagonals, making the stencil reproduction of the GEMM exact.
    """
    t = np.arange(S)
    W9 = np.zeros((S, NTAP), np.float32)
    for k, d in enumerate(DIAG_OFFSETS):
        s = t + d
        valid = (s >= 0) & (s < S)
        W9[valid, k] = adjacency[t[valid], s[valid]]
    exact = np.count_nonzero(adjacency) == np.count_nonzero(W9)
    return W9, exact


def _prep_dense_inmaps(sp_flat, E_flat, adjacency):
    spt = np.ascontiguousarray(sp_flat.T.reshape(KC, P, B).transpose(1, 0, 2))
    ef = np.ascontiguousarray(E_flat.reshape(KC, P).T)
    adj_bf = adjacency.astype(ml_dtypes.bfloat16)
    in_maps = []
    for m in range(NCORES):
        adjt_m = np.ascontiguousarray(adj_bf[m * TL : (m + 1) * TL, :].T)
        in_maps.append({"adjt": adjt_m, "spt": spt, "ef": ef})
    return in_maps


def _prep_sparse_inmaps(sp_flat, E_flat, W9):
    # fold the E-modulation into the tap weights: exact because the factor is
    # the power-of-two scale {1.0, -0.5}
    fac = 1.5 * E_flat - 0.5
    t = np.arange(S)
    wfold = np.empty_like(W9)  # [S, 9]
    for k, d in enumerate(DIAG_OFFSETS):
        s = np.clip(t + d, 0, S - 1)
        wfold[:, k] = W9[:, k] * fac[s]
    wfold = wfold.astype(np.float16)

    sp_pad = np.zeros((B, S + 2 * PADE + 8), np.float16)
    sp_pad[:, PADE : PADE + S] = sp_flat

    in_maps = []
    for m in range(NCORES):
        t0 = m * TL
        s0 = np.empty((NQ, B, SW), np.float16)
        s1 = np.empty((NQ, B, SW), np.float16)
        for q in range(NQ):
            tq = t0 + q * QT
            s0[q] = sp_pad[:, tq : tq + SW]
            s1[q] = sp_pad[:, tq + 1 : tq + 1 + SW]

        wslab = wfold[t0 : t0 + TL].reshape(NQ, QT, NTAP)
        im = {"s0": s0.reshape(P, SW), "s1": s1.reshape(P, SW)}
        for g, ks in enumerate(WGROUPS):
            wg = wslab[:, :, list(ks)].transpose(0, 2, 1)      # [NQ, |ks|, QT]
            wg = np.broadcast_to(wg[:, None], (NQ, B, len(ks), QT))
            im[f"wg{g}"] = np.ascontiguousarray(wg).reshape(P, len(ks), QT)
        in_maps.append(im)
    return in_maps


def _gather_out(results):
    out = np.empty((B, S), np.float32)
    for m in range(NCORES):
        r = results[m]["out"]
        if r.shape == (P, QT):  # sparse path: unpack [32q+b, t] -> [b, q*QT+t]
            r = r.astype(np.float32).reshape(NQ, B, QT).transpose(1, 0, 2)
            r = r.reshape(B, TL)
        out[:, m * TL : (m + 1) * TL] = r
    return out


def kernel(spikes, E, adjacency):
    spikes = np.asarray(spikes, np.float32)
    E = np.asarray(E, np.float32)
    adjacency = np.asarray(adjacency, np.float32)
    sp_flat = spikes.reshape(B, S)
    E_flat = E.reshape(S)

    W9, exact = _extract_diagonals(adjacency)
    if exact:
        in_maps = _prep_sparse_inmaps(sp_flat, E_flat, W9)
        results = _run(_get_prog("sparse"), in_maps).results
    else:
        in_maps = _prep_dense_inmaps(sp_flat, E_flat, adjacency)
        results = _run(_get_prog("dense"), in_maps).results
    return _gather_out(results).reshape(B, H, W)
